# revision 11
# baseline (speedup 1.0000x reference)
"""AdaMemNet SNN kernel for 8 TRN2 NeuronCores (Bass, SPMD data-parallel).

Problem: spikes [200, 32, 10000] f32 (0/1), W [3, 10000], b [3].
  proj = einsum('tbi,oi->tbo', spikes, W) + b
  then a 200-step adaptive-threshold LIF scan over t:
    mem = 0.99*mem + x; spk = (mem > thr); mem -= spk*thr
    thr = 0.95*thr + 5*spk
  returns (spk_rec, mem_rec), each [200, 32, 3].

Strategy (pure data parallel, no collectives):
  - Shard batch: 4 batch rows per core.
  - Host-side: transpose spikes shard to [i, tb] (tb = t*4+b), cast to
    bf16 (0/1 exact), pad i to 10240 (80 chunks of 128) with a bias row
    of ones at i=10000; tb padded 800 -> 896 (7 blocks of 128).
  - W is split into 3 bf16 pieces (hi/mid/lo); products with 0/1 spikes
    are exact, PSUM accumulates in fp32 -> f32-level precision at bf16
    matmul speed.
  - Per core: PE accumulates proj^T [9, 128] per tb-block (80 chunks),
    Pool combines the 3 pieces into projT [3, 896] (layout [o, t*4+b]),
    DVE runs the sequential scan with 5 fused ops per step, outputs
    stream out per block.
"""

import os
import sys

for _p in ("/opt/trn_rl_repo", "/opt/pypackages"):
    if _p not in sys.path:
        sys.path.insert(0, _p)

import numpy as np
import ml_dtypes

BF16 = ml_dtypes.bfloat16

# problem constants
T, B, NIN, NOUT = 200, 32, 10000, 3
NCORES = 8
BL = B // NCORES            # 4 batch rows per core
TB = T * BL                 # 800 real (t, b) rows per core
IC = 128                    # contraction chunk (partition dim)
NCH = 80                    # padded chunk count (10240 = 80*128)
IPAD = NCH * IC             # 10240 (row 10000 = bias ones row)
NPIECE = 3                  # bf16 split pieces of W
PCOL = 32                   # partition spacing of pieces (32-align rule)
M = PCOL * (NPIECE - 1) + NOUT  # 67 stationary columns (pieces at 0/32/64)
NB = 7                      # tb blocks of 128
BW = 128                    # tb block width
TBPAD = NB * BW             # 896
NGRP = 5                    # DMA group granularity in the DRAM layout
GRP = 16                    # chunks per group (5*16 = 80)
BETA, THR_INIT, SCALE, THR_DECAY = 0.99, 1.0, 5.0, 0.95

_CACHE = {}


def _build_nc():
    """Build the single-core Bass graph (same graph SPMD on all 8 cores)."""
    from contextlib import ExitStack

    import concourse.bass as bass
    import concourse.mybir as mybir

    fp32 = mybir.dt.float32
    bf16 = mybir.dt.bfloat16

    nc = bass.Bass()

    sp_ext = nc.declare_dram_parameter("sp", [NB, NGRP, IC, GRP, BW], bf16,
                                       isOutput=False)
    wp_ext = nc.declare_dram_parameter("wp", [IC, NCH, M], bf16, isOutput=False)
    spk_ext = nc.declare_dram_parameter("spk", [NOUT, TB], fp32, isOutput=True)
    mem_ext = nc.declare_dram_parameter("mem", [NOUT, TB], fp32, isOutput=True)

    ctx = ExitStack()
    with ctx:
        tiles = [
            ctx.enter_context(nc.sbuf_tensor(f"tile{i}", [IC, NCH, BW], bf16))
            for i in range(2)
        ]
        wp_sb = ctx.enter_context(nc.sbuf_tensor("wp_sb", [IC, NCH, M], bf16))
        projT = ctx.enter_context(nc.sbuf_tensor("projT", [NOUT, TBPAD], fp32))
        spk5T = ctx.enter_context(nc.sbuf_tensor("spk5T", [NOUT, TBPAD], fp32))
        spkoT = ctx.enter_context(nc.sbuf_tensor("spkoT", [NOUT, TBPAD], fp32))
        memT = ctx.enter_context(nc.sbuf_tensor("memT", [NOUT, TBPAD], fp32))
        u_buf = ctx.enter_context(nc.sbuf_tensor("u_buf", [NOUT, BL], fp32))
        v_buf = ctx.enter_context(nc.sbuf_tensor("v_buf", [NOUT, BL], fp32))
        thr_buf = ctx.enter_context(nc.sbuf_tensor("thr_buf", [NOUT, BL], fp32))
        zero_buf = ctx.enter_context(
            nc.sbuf_tensor("zero_buf", [NOUT, BL], fp32))
        comb3 = ctx.enter_context(nc.sbuf_tensor("comb3", [NOUT, BW], fp32))
        psums = [
            ctx.enter_context(nc.psum_tensor(f"psum{i}", [M, BW], fp32))
            for i in range(2)
        ]

        dsems = [
            ctx.enter_context(nc.semaphore(f"dma_sem{b}")) for b in range(NB)
        ]
        with (
            nc.Block() as block,
            nc.semaphore("wdma_sem") as wdma_sem,   # wp DMA
            nc.semaphore("pe_sem") as pe_sem,       # PE done with block b
            nc.semaphore("pool_sem") as pool_sem,   # projT block b ready
            nc.semaphore("dve_sem") as dve_sem,     # scan block b done
            nc.semaphore("odma_sem") as odma_sem,   # output DMAs
        ):

            @block.sync
            def _(sync: bass.BassEngine):
                sync.dma_start(out=wp_sb[:, :, :], in_=wp_ext[:, :, :]).then_inc(
                    wdma_sem, 16)
                for b in range(NB):
                    if b >= 2:
                        # tile buffer reuse: PE must be done with block b-2
                        sync.wait_ge(pe_sem, b - 1)
                    tile = tiles[b % 2]
                    for g in range(NGRP):
                        sync.dma_start(
                            out=tile[:, g * GRP:(g + 1) * GRP, :],
                            in_=sp_ext[b, g, :, :, :],
                        ).then_inc(dsems[b], 16)

            @block.tensor
            def _(pe: bass.BassEngine):
                pe.wait_ge(wdma_sem, 16)
                for b in range(NB):
                    tile = tiles[b % 2]
                    psum = psums[b % 2]
                    if b >= 2:
                        # psum bank reuse: Pool must have combined block b-2
                        pe.wait_ge(pool_sem, b - 1)
                    pe.wait_ge(dsems[b], 16 * NGRP)
                    for c in range(NCH):
                        mm = pe.matmul(
                            psum[:, :],
                            wp_sb[:, c, :],
                            tile[:, c, :],
                            start=(c == 0),
                            stop=(c == NCH - 1),
                        )
                        if c == NCH - 1:
                            mm.then_inc(pe_sem, 1)

            @block.vector
            def _(dve: bass.BassEngine):
                dve.memset(zero_buf[:, :], 0.0)
                dve.memset(thr_buf[:, :], THR_INIT)
                dve.drain()
                for b in range(NB):
                    psum = psums[b % 2]
                    off = b * BW
                    nsteps = min(T - b * (BW // BL), BW // BL)  # 32 (8 last)
                    dve.wait_ge(pe_sem, b + 1)
                    # proj = piece0 + piece1 + piece2 (pieces at partition
                    # 0/32/64; never two PSUM operands in one op). DVE does
                    # not interlock same-engine RAW: drain between dep ops.
                    dve.tensor_copy(comb3[:, :], psum[PCOL:PCOL + NOUT, :])
                    dve.drain()
                    dve.tensor_tensor(
                        out=comb3[:, :], in0=psum[0:NOUT, :], in1=comb3[:, :],
                        op=mybir.AluOpType.add)
                    dve.drain()
                    dve.tensor_tensor(
                        out=projT[:, off:off + BW],
                        in0=psum[2 * PCOL:2 * PCOL + NOUT, :],
                        in1=comb3[:, :], op=mybir.AluOpType.add,
                    ).then_inc(pool_sem, 1)
                    dve.drain()
                    for tl in range(nsteps):
                        t = b * (BW // BL) + tl
                        col = slice(4 * t, 4 * t + 4)
                        prev = zero_buf[:, :] if t == 0 else memT[:, 4*t-4:4*t]
                        # m = 0.99*mem + x   (pre-reset, into memT)
                        dve.scalar_tensor_tensor(
                            out=memT[:, col], in0=prev, scalar=BETA,
                            in1=projT[:, col],
                            op0=mybir.AluOpType.mult, op1=mybir.AluOpType.add)
                        dve.drain()
                        # u = m - thr
                        dve.tensor_tensor(
                            out=u_buf[:, :], in0=memT[:, col], in1=thr_buf[:, :],
                            op=mybir.AluOpType.subtract)
                        dve.drain()
                        # v = (u > 0) * thr   (reset amount, exact)
                        dve.scalar_tensor_tensor(
                            out=v_buf[:, :], in0=u_buf[:, :], scalar=0.0,
                            in1=thr_buf[:, :],
                            op0=mybir.AluOpType.is_gt, op1=mybir.AluOpType.mult)
                        # s5 = (u > 0) * 5
                        dve.tensor_scalar(
                            out=spk5T[:, col], in0=u_buf[:, :], scalar1=0.0,
                            scalar2=SCALE, op0=mybir.AluOpType.is_gt,
                            op1=mybir.AluOpType.mult)
                        dve.drain()
                        # mem = m - v
                        dve.tensor_tensor(
                            out=memT[:, col], in0=memT[:, col], in1=v_buf[:, :],
                            op=mybir.AluOpType.subtract)
                        # thr = 0.95*thr + s5
                        dve.scalar_tensor_tensor(
                            out=thr_buf[:, :], in0=thr_buf[:, :],
                            scalar=THR_DECAY, in1=spk5T[:, col],
                            op0=mybir.AluOpType.mult, op1=mybir.AluOpType.add)
                        dve.drain()
                    # spk = s5 * 0.2  (exact: fl(5*0.2f) == 1.0f)
                    wr = min(TB - off, BW)
                    dve.tensor_scalar(
                        out=spkoT[:, off:off + wr], in0=spk5T[:, off:off + wr],
                        scalar1=0.2, scalar2=None,
                        op0=mybir.AluOpType.mult,
                    ).then_inc(dve_sem, 1)
                    dve.drain()

            @block.scalar
            def _(act: bass.BassEngine):
                ndma = 0
                for b in range(NB):
                    off = b * BW
                    wr = min(TB - off, BW)  # 128, last block 32
                    act.wait_ge(dve_sem, b + 1)
                    act.dma_start(
                        out=spk_ext[:, off:off + wr],
                        in_=spkoT[:, off:off + wr],
                    ).then_inc(odma_sem, 16)
                    act.dma_start(
                        out=mem_ext[:, off:off + wr],
                        in_=memT[:, off:off + wr],
                    ).then_inc(odma_sem, 16)
                    ndma += 2
                act.wait_ge(odma_sem, 16 * ndma)

    return nc


def _split_w_pieces(wt_pad: np.ndarray) -> np.ndarray:
    """Split f32 [IPAD, NOUT] into NPIECE bf16 pieces -> [IPAD, M] f32-exactish.

    Layout: piece p occupies columns [32p, 32p+3).
    """
    out = np.zeros((IPAD, M), dtype=BF16)
    resid = wt_pad.astype(np.float32).copy()
    for p in range(NPIECE):
        piece = resid.astype(BF16)
        out[:, PCOL * p:PCOL * p + NOUT] = piece
        resid = resid - piece.astype(np.float32)
    return out


def _prep_inputs(spikes: np.ndarray, W: np.ndarray, b: np.ndarray):
    """Host-side shard prep: per-core transposed bf16 spikes + W pieces."""
    spikes = np.asarray(spikes, dtype=np.float32)
    W = np.asarray(W, dtype=np.float32)
    b = np.asarray(b, dtype=np.float32)

    # Wt padded: rows 0..9999 = W.T, row 10000 = bias, rest zero
    wt_pad = np.zeros((IPAD, NOUT), dtype=np.float32)
    wt_pad[:NIN] = W.T
    wt_pad[NIN] = b
    wp = _split_w_pieces(wt_pad)                      # [IPAD, 9] bf16
    # partition-major: wp_pm[p, c, j] = wp[c*128 + p, j]
    wp_pm = np.ascontiguousarray(
        wp.reshape(NCH, IC, M).transpose(1, 0, 2))    # [128, 80, 9]

    # spikes -> [i, t, b] once (biggest host cost)
    sp_itb = np.ascontiguousarray(spikes.transpose(2, 0, 1))  # [10000, 200, 32]

    in_maps = []
    for c in range(NCORES):
        arr = np.zeros((IPAD, TBPAD), dtype=BF16)
        sl = sp_itb[:, :, 4 * c:4 * c + 4].reshape(NIN, TB)   # [10000, 800]
        arr[:NIN, :TB] = sl                                    # exact 0/1 cast
        arr[NIN, :TB] = BF16(1.0)                              # bias ones row
        # [IPAD, TBPAD] -> [g, q, p, blk, w] -> [blk, g, p, q, w]
        v = arr.reshape(NGRP, GRP, IC, NB, BW).transpose(3, 0, 2, 1, 4)
        sp_arr = np.ascontiguousarray(v)
        in_maps.append({"sp": sp_arr, "wp": wp_pm})
    return in_maps


def kernel(spikes: np.ndarray, W: np.ndarray, b: np.ndarray, *, trace=False):
    from concourse.bass_utils import run_bass_kernel_spmd

    if "nc" not in _CACHE:
        _CACHE["nc"] = _build_nc()
    nc = _CACHE["nc"]

    in_maps = _prep_inputs(spikes, W, b)
    res = run_bass_kernel_spmd(nc, in_maps, core_ids=list(range(NCORES)),
                               trace=trace)
    spk_full = np.empty((T, B, NOUT), dtype=np.float32)
    mem_full = np.empty((T, B, NOUT), dtype=np.float32)
    for c in range(NCORES):
        spk_full[:, 4 * c:4 * c + 4, :] = res.results[c]["spk"].T.reshape(
            T, BL, NOUT)
        mem_full[:, 4 * c:4 * c + 4, :] = res.results[c]["mem"].T.reshape(
            T, BL, NOUT)
    kernel.last_exec_time_ns = res.exec_time_ns
    return spk_full, mem_full


kernel.last_exec_time_ns = None

if __name__ == "__main__":
    # smoke test with random data
    rng = np.random.default_rng(0)
    spikes = (rng.random((T, B, NIN)) < rng.random((B, NIN))).astype(np.float32)
    W = (rng.standard_normal((NOUT, NIN)) * 0.01).astype(np.float32)
    b = (rng.standard_normal(NOUT) * 0.01).astype(np.float32)
    spk, mem = kernel(spikes, W, b)
    print("spk mean:", spk.mean(), "mem mean:", mem.mean())
